# revision 56
# baseline (speedup 1.0000x reference)
"""Trainium2 Bass kernel for the LDE guided-attention module.

Sharding: 8 cores = 2 samples x 4 row-quarters of the N=9216 attention rows.
Each core redundantly computes the (cheap) conv trunk for its sample's
quarter (+halo), then its quarter of softmax(d1@d2)@c1 flash-attention
style -- the [N,N] map never leaves PSUM/SBUF.  d2 (keys) and c1 (values)
quarters are exchanged with the 3 sibling cores via AllGather.

Attention inner loop is software-pipelined two groups deep: scores are
issued to the PE queue two exp-periods before their guided-accumulation
matmuls, so the exp() stream on the scalar engine never waits on scores
and the guided matmuls never wait on exp.  Score groups are 3 key-tiles
wide (one [128, 3*rb] PSUM tile, double-buffered) so each exp covers 1536
columns, amortizing the per-instruction ACT overhead.

All matmul operands are bf16 (fp32 operands stream at half rate; scores
max |s| ~ 0.58 so bf16 is safe).  Dependency-free filler LDWEIGHTS keep
the PE array active through exp-wait gaps: an idle PE trips the HAM
activity monitor, which halves the PE clock (4/8 gate) and was measured
to cost ~30% end to end.

PSUM budget (16KB/partition): tag "pss" 2 x 6KB (score groups) +
tag "psacc" 2 x 2KB (guided accumulator ring, also trunk/epilogue scratch).
"""

import sys

for _p in ("/opt/trn_rl_repo",):
    if _p not in sys.path:
        sys.path.insert(0, _p)

import numpy as np

import concourse.bass as bass
import concourse.bacc as bacc
import concourse.mybir as mybir
from concourse import tile
from concourse.bass_utils import run_bass_kernel_spmd

F32 = mybir.dt.float32
F32R = mybir.dt.float32r
BF16 = mybir.dt.bfloat16
AF = mybir.ActivationFunctionType

C = 64          # channels
CQ = 32         # C // 2
H = W = 96
N = H * W       # 9216
NT = N // 128   # 72 column tiles
GRP = 3         # key tiles per score group
NG = NT // GRP  # 24 groups
LAG = 8         # guided-matmul retirement lag (groups) behind scores
QROWS = 24      # image rows per quarter
NQ = QROWS * W  # 2304 attention rows per core
PW = 98         # padded width
CHUNK_ROWS = 4
CHUNK = CHUNK_ROWS * W  # 384
BLOCKS = [(0, 512), (512, 512), (1024, 512), (1536, 512), (2048, 256)]
# packed weight buffer column offsets
W1O, W2O, C1O, C2O, C3O, C4O, B1O, B2O = 0, 576, 640, 672, 704, 736, 800, 802
WPC = 804

_cache = {}


def _r(ap):
    return ap


def _build(a1: float, a2: float, loop_n: int = 1, single_core: bool = False):
    nc = bacc.Bacc(None, target_bir_lowering=False)
    xrq = nc.declare_dram_parameter("xrq", [C, 26 * PW], BF16, isOutput=False)
    xdq = nc.declare_dram_parameter("xdq", [C, 26 * PW], BF16, isOutput=False)
    wpack = nc.declare_dram_parameter("wpack", [C, WPC], BF16, isOutput=False)
    bpack = nc.declare_dram_parameter("bpack", [C, 4], F32, isOutput=False)
    out = nc.declare_dram_parameter("out", [C, NQ], F32, isOutput=True)
    GROUPS = [[0, 1, 2, 3], [4, 5, 6, 7]]
    HQ = NQ // 2  # 1152
    d2b1 = nc.dram_tensor("d2b1", [CQ, HQ], BF16)
    d2g1 = nc.dram_tensor("d2g1", [4, CQ, HQ], BF16)
    d2b2 = nc.dram_tensor("d2b2", [CQ, HQ], BF16)
    d2g2 = nc.dram_tensor("d2g2", [4, CQ, HQ], BF16)
    HT = (NT // 8) * (CQ + 1)  # 9 tiles * 33
    c1b1 = nc.dram_tensor("c1b1", [128, HT], BF16)
    c1g1 = nc.dram_tensor("c1g1", [4, 128, HT], BF16)
    c1b2 = nc.dram_tensor("c1b2", [128, HT], BF16)
    c1g2 = nc.dram_tensor("c1g2", [4, 128, HT], BF16)

    with tile.TileContext(nc) as tc:
        with (
            tc.tile_pool(name="const", bufs=1) as cpool,
            tc.tile_pool(name="xpad", bufs=1) as xpool,
            tc.tile_pool(name="big", bufs=1) as bpool,
            tc.tile_pool(name="chunk", bufs=3) as kpool,
            tc.tile_pool(name="pt", bufs=3) as ptpool,
            tc.tile_pool(name="ep", bufs=2) as eppool,
            tc.tile_pool(name="ps", bufs=2, space="PSUM") as ps,
        ):
            # ---- constants: one packed DMA; ones via memset ----
            wp_sb = cpool.tile([C, WPC], BF16)
            nc.sync.dma_start(wp_sb[:], wpack[:])
            w1t_sb = wp_sb[:, W1O:W1O + 9 * C]
            w2t_sb = wp_sb[:, W2O:W2O + C]
            wch1t_sb = wp_sb[:, C1O:C1O + CQ]
            wch2t_sb = wp_sb[:, C2O:C2O + CQ]
            wch3t_sb = wp_sb[:, C3O:C3O + CQ]
            wch4t_sb = wp_sb[0:CQ, C4O:C4O + C]
            bp_sb = cpool.tile([C, 4], F32)
            nc.sync.dma_start(bp_sb[:], bpack[:])
            b1_sb = bp_sb[:, 0:2]
            b2_sb = bp_sb[:, 2:4]
            ones_sb = cpool.tile([1, C], BF16)
            nc.vector.memset(ones_sb[:], 1.0)

            import contextlib
            loop_cm = tc.For_i(0, loop_n, 1) if loop_n > 1 else \
                contextlib.nullcontext()
            with loop_cm:
                _body(nc, tc, locals())

    nc.finalize()
    return nc


def _body(nc, tc, env):
    (cpool, xpool, bpool, kpool, ptpool, eppool, ps) = (
        env[k] for k in ("cpool", "xpool", "bpool", "kpool", "ptpool",
                         "eppool", "ps"))
    (w1t_sb, w2t_sb, wch1t_sb, wch2t_sb, wch3t_sb, wch4t_sb, b1_sb, b2_sb,
     ones_sb) = (env[k] for k in ("w1t_sb", "w2t_sb", "wch1t_sb", "wch2t_sb",
                                  "wch3t_sb", "wch4t_sb", "b1_sb", "b2_sb",
                                  "ones_sb"))
    (xrq, xdq, out, a1, a2, d2b1, d2g1, d2b2, d2g2, c1b1, c1g1, c1b2, c1g2,
     GROUPS, HQ) = (
        env[k] for k in ("xrq", "xdq", "out", "a1", "a2", "d2b1", "d2g1",
                         "d2b2", "d2g2", "c1b1", "c1g1", "c1b2", "c1g2",
                         "GROUPS", "HQ"))

    # ---- persistent intermediates ----
    d2_sb = bpool.tile([CQ, N], BF16)           # scores lhsT source
    c1aug = bpool.tile([128, NT, CQ + 1], BF16)  # c1 N-major + ones col
    d1q = bpool.tile([CQ, NQ], BF16)
    d0q = bpool.tile([C, NQ], BF16)

    # ---- trunk: depth first (d2 gather overlaps the rgb trunk that
    # follows), then rgb (c1 gathered in two half-chains, the first
    # launched mid-trunk). ----
    # slabs arrive pre-padded from the host (26 x 98 with zero edge cols)
    # so the HBM->SBUF DMA is one contiguous run per partition.
    NTQ = NT // 4  # 18 tiles per quarter
    d2q_sb = bpool.tile([CQ, NQ], BF16)
    c1q_sb = bpool.tile([128, NTQ, CQ + 1], BF16)  # local c1 + ones col
    rq_slab = xpool.tile([C, 26, PW], BF16, tag="rqslab")
    nc.sync.dma_start(rq_slab[:], xrq[:].rearrange("c (r w) -> c r w", w=PW))
    dq_slab = xpool.tile([C, 26, PW], BF16, tag="dqslab")
    nc.sync.dma_start(dq_slab[:], xdq[:].rearrange("c (r w) -> c r w", w=PW))
    nc.vector.memset(c1q_sb[:, :, CQ:CQ + 1], 1.0)

    def conv1_stage(slab, j):
        psc = ps.tile([C, CHUNK], F32, tag="pss", name="psc")
        for k in range(9):
            ky, kx = divmod(k, 3)
            rhs = slab[:, 4 * j + ky: 4 * j + ky + CHUNK_ROWS, kx: kx + W]
            nc.tensor.matmul(psc[:], _r(w1t_sb[:, k * C:(k + 1) * C]),
                             _r(rhs), start=(k == 0), stop=(k == 8))
        return psc

    def mid_fin_stage(psc, out_ap):
        pre = kpool.tile([C, CHUNK], BF16, tag="tp", name="pre")
        nc.scalar.activation(pre[:], psc[:], AF.Prelu, bias=b1_sb[:, 0:1],
                             alpha=a1)
        psc2 = ps.tile([C, CHUNK], F32, tag="psacc", name="psc2")
        nc.tensor.matmul(psc2[:], _r(w2t_sb[:]), _r(pre[:]),
                         start=True, stop=True)
        nc.scalar.activation(out_ap, psc2[:], AF.Prelu, bias=b2_sb[:, 0:1],
                             alpha=a2)

    def proj_d(i):
        sl = slice(i * CHUNK, (i + 1) * CHUNK)
        psq = ps.tile([CQ, CHUNK], F32, tag="psacc", name="psq")
        nc.tensor.matmul(psq[:], _r(wch2t_sb[:]), _r(d0q[:, sl]),
                         start=True, stop=True)
        psd = ps.tile([CQ, CHUNK], F32, tag="psacc", name="psd")
        nc.tensor.matmul(psd[:], _r(wch3t_sb[:]), _r(d0q[:, sl]),
                         start=True, stop=True)
        nc.vector.tensor_copy(d1q[:, sl], psq[:])
        nc.vector.tensor_copy(d2q_sb[:, sl], psd[:])
        for _ in range(2):
            nc.tensor.ldweights(_r(w1t_sb[:, 0:128]))

    def proj_r(i, cc):
        psns = {}
        for ii in range(3):
            psn = ps.tile([128, CQ], F32, tag="psacc", name="psn")
            nc.tensor.matmul(psn[:], _r(cc[:, ii * 128:(ii + 1) * 128]),
                             _r(wch1t_sb[:]), start=True, stop=True)
            psns[ii] = psn
            if ii >= 1:
                nc.vector.tensor_copy(
                    c1q_sb[:, 3 * i + ii - 1, 0:CQ], psns.pop(ii - 1))
        nc.vector.tensor_copy(c1q_sb[:, 3 * i + 2, 0:CQ], psns.pop(2))
        for _ in range(2):
            nc.tensor.ldweights(_r(w1t_sb[:, 0:128]))

    NCH = NQ // CHUNK  # 6
    # Key tiles are laid out half-interleaved: physical tile position
    # p<36 holds quarter p//9 local-tile p%9 (each quarter's first 9),
    # p>=36 the quarters' last 9.  Attention consumes positions in
    # order, so each half-gather feeds a contiguous run of groups.
    # All four collectives are triggered as soon as their source half is
    # ready (d2h1 mid-depth-trunk); they execute strictly in trigger
    # order on the collective engine.  The SBUF fills are emitted LAST
    # on the SP queue, in collective-completion order, so a fill waiting
    # on a late collective never delays an earlier consumer.
    def gather(cbuf, gbuf, src_ap):
        nc.sync.dma_start(cbuf[:], src_ap)
        if env.get("single_core"):
            for g in range(4):
                nc.gpsimd.dma_start(gbuf[g], cbuf[:])
        else:
            nc.gpsimd.collective_compute(
                "AllGather", mybir.AluOpType.bypass,
                replica_groups=GROUPS, ins=[cbuf[:]], outs=[gbuf[:]])

    # depth trunk, pipelined two chunks deep; d2 half-gathers launch the
    # moment half of d2q_sb exists.
    pscs = {0: conv1_stage(dq_slab, 0), 1: conv1_stage(dq_slab, 1)}
    for j in range(NCH):
        sl = slice(j * CHUNK, (j + 1) * CHUNK)
        mid_fin_stage(pscs.pop(j), d0q[:, sl])
        if j + 2 < NCH:
            pscs[j + 2] = conv1_stage(dq_slab, j + 2)
        if j >= 1:
            proj_d(j - 1)
            if j - 1 == 2:
                gather(d2b1, d2g1, d2q_sb[:, 0:HQ])
    proj_d(NCH - 1)
    gather(d2b2, d2g2, d2q_sb[:, HQ:NQ])

    # rgb trunk, same pipeline; c1 half-gathers launch mid-trunk/end.
    ccs = {}
    pscs = {0: conv1_stage(rq_slab, 0), 1: conv1_stage(rq_slab, 1)}
    for j in range(NCH):
        cc = kpool.tile([C, CHUNK], BF16, tag="tc", name="cc")
        ccs[j] = cc
        mid_fin_stage(pscs.pop(j), cc[:])
        if j + 2 < NCH:
            pscs[j + 2] = conv1_stage(rq_slab, j + 2)
        if j >= 1:
            proj_r(j - 1, ccs.pop(j - 1))
            if j - 1 == 2:
                gather(c1b1, c1g1, c1q_sb[:, 0:9, :])
    proj_r(NCH - 1, ccs.pop(NCH - 1))
    gather(c1b2, c1g2, c1q_sb[:, 9:18, :])

    # fills, emitted last, in collective-completion order
    for h, gbuf in ((0, d2g1), (1, d2g2)):
        for g in range(4):
            nc.sync.dma_start(
                d2_sb[:, (36 * h + 9 * g) * 128:(36 * h + 9 * g + 9) * 128],
                gbuf[g])
    for h, gbuf in ((0, c1g1), (1, c1g2)):
        for g in range(4):
            nc.sync.dma_start(
                c1aug[:, 36 * h + 9 * g:36 * h + 9 * g + 9, :],
                gbuf[g].rearrange("p (t q) -> p t q", q=CQ + 1))

    # ---- streaming attention: one flat stream of (block, group) items,
    # software-pipelined two deep (S two groups ahead of exp, exp one
    # group ahead of guided) so the ACT exp stream never waits. ----
    items = []
    for bi, (o, rb) in enumerate(BLOCKS):
        grp = (GRP * 512) // rb      # 3 key tiles at rb=512, 6 at rb=256
        for g in range(NT // grp):
            items.append((bi, o, rb, g, grp))
    NI = len(items)
    scs = {}
    pts = {}
    accs = {}
    ep_parts = {}

    def s_group(i):
        bi, o, rb, g, grp = items[i]
        ps_sc = ps.tile([128, grp * rb], F32, tag="pss", name="ps_sc")
        for ii in range(grp):
            t = grp * g + ii
            nc.tensor.matmul(ps_sc[:, ii * rb:(ii + 1) * rb],
                             _r(d2_sb[:, t * 128:(t + 1) * 128]),
                             _r(d1q[:, o:o + rb]), start=True, stop=True)
        scs[i] = ps_sc

    def do_exp(i):
        bi, o, rb, g, grp = items[i]
        pT = ptpool.tile([128, grp * rb], BF16, tag="pt", name="pT",
                         bufs=LAG + 1)
        nc.scalar.activation(pT[:], scs.pop(i)[:], AF.Exp)
        pts[i] = pT

    def do_guided(i):
        bi, o, rb, g, grp = items[i]
        if g == 0:
            accs[bi] = ps.tile([CQ + 1, rb], F32, tag="psacc", name="acc")
        pT = pts.pop(i)
        for ii in range(grp):
            t = grp * g + ii
            nc.tensor.matmul(accs[bi][:], _r(c1aug[:, t, :]),
                             _r(pT[:, ii * rb:(ii + 1) * rb]),
                             start=(t == 0), stop=(t == NT - 1),
                             skip_group_check=True)

    def ep_a(bi):
        o, rb = BLOCKS[bi]
        acc = accs.pop(bi)
        g_sb = eppool.tile([CQ, rb], BF16, tag="gsb", name="g_sb")
        nc.vector.tensor_copy(g_sb[:], acc[0:CQ, :])
        sum_sb = eppool.tile([1, rb], BF16, tag="ssb", name="sum_sb")
        nc.vector.tensor_copy(sum_sb[:], acc[CQ:CQ + 1, :])
        ps_b = ps.tile([C, rb], F32, tag="psacc", name="ps_b")
        nc.tensor.matmul(ps_b[:], _r(ones_sb[:]), _r(sum_sb[:]),
                         start=True, stop=True)
        rcp = eppool.tile([C, rb], F32, tag="rcp", name="rcp")
        nc.vector.reciprocal(rcp[:], ps_b[:])
        ep_parts[bi] = (o, rb, g_sb, rcp)

    def ep_b(bi):
        o, rb, g_sb, rcp = ep_parts.pop(bi)
        ps_o = ps.tile([C, rb], F32, tag="psacc", name="ps_o")
        nc.tensor.matmul(ps_o[:], _r(wch4t_sb[:]), _r(g_sb[:]),
                         start=True, stop=True)
        # padding allocation: keeps the psacc ring at 4 slots/block so the
        # next block's accumulator reuses the (quickly freed) acc slot
        # instead of ps_b's slot, whose reader is the 3.3us reciprocal.
        ps.tile([C, rb], F32, tag="psacc", name="ps_pad")
        o1 = eppool.tile([C, rb], F32, tag="o1", name="o1")
        nc.vector.tensor_mul(o1[:], ps_o[:], rcp[:])
        osb = eppool.tile([C, rb], F32, tag="osb", name="osb")
        nc.vector.tensor_add(osb[:], o1[:], d0q[:, o:o + rb])
        nc.sync.dma_start(out[:, o:o + rb], osb[:])

    ep_b_pend = []

    def retire(j):
        do_guided(j)
        bj, _, _, gj, grpj = items[j]
        if gj == NT // grpj - 1:
            ep_a(bj)
            ep_b_pend.append(bj)

    for i in range(NI):
        s_group(i)
        while ep_b_pend:
            ep_b(ep_b_pend.pop())
        if i >= 1:
            do_exp(i - 1)
        if i >= LAG:
            retire(i - LAG)
        # dependency-free filler LDWEIGHTS: keep the PE array active
        # through the exp-wait gap so the HAM clock gate stays at 8/8
        # (idle-throttled PE would run every matmul at half clock).
        for _ in range(3):
            nc.tensor.ldweights(_r(d2_sb[:, 0:128]))
    do_exp(NI - 1)
    for j in range(NI - LAG, NI):
        while ep_b_pend:
            ep_b(ep_b_pend.pop())
        retire(j)
    while ep_b_pend:
        ep_b(ep_b_pend.pop())


def _prep_inputs(rgb, depth, w1, b1, a1, w2, b2, a2, wch1, wch2, wch3, wch4):
    import ml_dtypes
    bf16 = ml_dtypes.bfloat16
    rgb = np.asarray(rgb, np.float32)
    depth = np.asarray(depth, np.float32)
    wp = np.zeros((C, WPC), np.float32)
    # w1t[ci, (ky*3+kx)*C + co]
    wp[:, W1O:W1O + 9 * C] = np.transpose(
        np.asarray(w1, np.float32), (1, 2, 3, 0)).reshape(C, 9 * C)
    wp[:, W2O:W2O + C] = np.asarray(w2, np.float32)[:, :, 0, 0].T
    wp[:, C1O:C1O + CQ] = np.asarray(wch1, np.float32)[:, :, 0, 0].T
    wp[:, C2O:C2O + CQ] = np.asarray(wch2, np.float32)[:, :, 0, 0].T
    wp[:, C3O:C3O + CQ] = np.asarray(wch3, np.float32)[:, :, 0, 0].T
    wp[0:CQ, C4O:C4O + C] = np.asarray(wch4, np.float32)[:, :, 0, 0].T
    wp = np.ascontiguousarray(wp.astype(bf16))
    bp = np.zeros((C, 4), np.float32)
    bp[:, 0] = bp[:, 1] = np.asarray(b1, np.float32)
    bp[:, 2] = bp[:, 3] = np.asarray(b2, np.float32)
    a1f = float(np.asarray(a1)); a2f = float(np.asarray(a2))

    in_maps = []
    for core in range(8):
        s, q = divmod(core, 4)
        xdq = np.zeros((C, 26, PW), np.float32)
        xrq = np.zeros((C, 26, PW), np.float32)
        for r_slab in range(26):
            r_img = q * QROWS - 1 + r_slab
            if 0 <= r_img < H:
                xdq[:, r_slab, 1:W + 1] = depth[s, :, r_img, :]
                xrq[:, r_slab, 1:W + 1] = rgb[s, :, r_img, :]
        in_maps.append({
            "xrq": np.ascontiguousarray(xrq.reshape(C, 26 * PW).astype(bf16)),
            "xdq": np.ascontiguousarray(xdq.reshape(C, 26 * PW).astype(bf16)),
            "wpack": wp,
            "bpack": bp,
        })
    return in_maps, (a1f, a2f)


def kernel(rgb, depth, w1, b1, a1, w2, b2, a2, wch1, wch2, wch3, wch4,
           _loop_n=1, **run_kwargs):
    in_maps, (a1f, a2f) = _prep_inputs(rgb, depth, w1, b1, a1, w2, b2, a2,
                                       wch1, wch2, wch3, wch4)
    key = (a1f, a2f, _loop_n)
    if key not in _cache:
        _cache[key] = _build(a1f, a2f, loop_n=_loop_n)
    nc = _cache[key]
    res = run_bass_kernel_spmd(nc, in_maps, list(range(8)), **run_kwargs)
    out_full = np.empty((2, C, H, W), np.float32)
    for core in range(8):
        s, q = divmod(core, 4)
        out_full[s, :, q * QROWS:(q + 1) * QROWS, :] = \
            res.results[core]["out"].reshape(C, QROWS, W)
    if run_kwargs:
        return out_full, res
    return out_full
